# revision 8
# baseline (speedup 1.0000x reference)
"""AdderNet CNN (nn_CNN_73306501808283) on 8 Trainium2 NeuronCores.

Data-parallel over batch (64 -> 8 per core). Full-batch BN statistics via
two tiny in-kernel AllReduces. Final channel-sum + negate + BN3 on host.

Per-core layout:
  conv1: 30 rounds of k-pairs; partitions = (ks=2, j=8, b=8) = 128.
    X_shift[(ks,j,b), i] = x[b, i+r(k), j+c(k)], k = 2t+ks, k = r*5+c.
    A = |X_shift - w1[co,k]|  (ScalarE Abs-with-bias / VectorE tensor_scalar)
    k-sum on TensorE: selector lhsT[128,64] passes (j,b) through, PSUM
    accumulates over 30 rounds -> psum_co[64=(j,b), 488=i] (POSITIVE sums;
    real conv out = -psum).
  BN sums fused into PSUM evacuation via activation(accum_out); pools are
  MIN-pools on the raw positive sums (BN scale > 0 because g == 1, and conv
  out = -sum, so the affine+relu is monotonically decreasing in the raw sum).
"""

import sys
from contextlib import ExitStack

import numpy as np

if "/opt/trn_rl_repo" not in sys.path:
    sys.path.insert(0, "/opt/trn_rl_repo")

import concourse.bass as bass
import concourse.tile as tile
from concourse import bacc, mybir
from concourse import bass_utils

F32 = mybir.dt.float32
I32 = mybir.dt.int32
AF = mybir.ActivationFunctionType
OP = mybir.AluOpType
AX = mybir.AxisListType

N_CORES = 8
B = 8          # images per core
H, W = 499, 12
KW = 5
C1 = 5         # conv1 out channels
HO, WO = 488, 8
K1 = 60
NROUND = 30    # k pairs
BLK = 3        # rounds per X_shift/A block
C2 = 10
K2H = 5
H2I = 122      # H after pool1
H2O = 118
NP1 = 64 * HO * WO
NP2 = 64 * H2O * 2
BN_EPS = 1e-5


def _ap(t_ap, offset, dims):
    return bass.AP(tensor=t_ap.tensor, offset=offset, ap=[list(d) for d in dims])


def build_kernel():
    nc = bacc.Bacc(
        "TRN2",
        target_bir_lowering=False,
        debug=False,
        enable_asserts=True,
        num_devices=N_CORES,
    )

    # x is host-transposed to [b, w, h]; w1p/w2p/wfcp are host-permuted tables
    x_d = nc.dram_tensor("x", [B * W * H], F32, kind="ExternalInput").ap()
    w1_d = nc.dram_tensor("w1p", [2 * NROUND * C1], F32, kind="ExternalInput").ap()
    g1_d = nc.dram_tensor("g1", [C1], F32, kind="ExternalInput").ap()
    b1_d = nc.dram_tensor("b1", [C1], F32, kind="ExternalInput").ap()
    w2_d = nc.dram_tensor("w2p", [C1 * C2 * K2H], F32, kind="ExternalInput").ap()
    g2_d = nc.dram_tensor("g2", [C2], F32, kind="ExternalInput").ap()
    b2_d = nc.dram_tensor("b2", [C2], F32, kind="ExternalInput").ap()
    wfc_d = nc.dram_tensor("wfcp", [C2 * C2 * 59], F32, kind="ExternalInput").ap()
    out_d = nc.dram_tensor("out", [80, C2], F32, kind="ExternalOutput").ap()

    with tile.TileContext(nc) as tc, ExitStack() as ctx:
        singles = ctx.enter_context(tc.tile_pool(name="singles", bufs=1))
        xpool = ctx.enter_context(tc.tile_pool(name="xshift", bufs=3))
        apool = ctx.enter_context(tc.tile_pool(name="adiff", bufs=8))
        psA = ctx.enter_context(tc.tile_pool(name="psA", bufs=1, space="PSUM"))
        psB = ctx.enter_context(tc.tile_pool(name="psB", bufs=1, space="PSUM"))
        scratch = ctx.enter_context(tc.tile_pool(name="scratch", bufs=2))
        dram = ctx.enter_context(tc.tile_pool(name="dram", bufs=1, space="DRAM"))

        # ---------------- one-time setup ----------------
        # conv1 bias tables: wpos[p=(ks,j,b), t, co] = w1[co, 2t+ks]
        wpos = singles.tile([128, NROUND, C1], F32)
        for ks in range(2):
            nc.sync.dma_start(
                out=wpos[ks * 64:(ks + 1) * 64, :, :],
                in_=_ap(w1_d, ks * NROUND * C1, [[0, 64], [1, NROUND * C1]]),
            )
        wneg = singles.tile([128, NROUND, C1], F32)
        nc.vector.tensor_scalar_mul(wneg[:], wpos[:], -1.0)

        # conv2 tables: p=(b,w,ci) 80 partitions; free (c2, dr)
        w2pos = singles.tile([128, C2, K2H], F32)
        nc.sync.dma_start(
            out=w2pos[0:80, :, :],
            in_=_ap(w2_d, 0, [[0, 16], [C2 * K2H, C1], [1, C2 * K2H]]),
        )
        w2neg = singles.tile([128, C2, K2H], F32)
        nc.vector.tensor_scalar_mul(w2neg[0:80], w2pos[0:80], -1.0)

        # fc weights: p=(b,c2) 80; free (c3, r)
        wfcr = singles.tile([128, C2, 59], F32)
        nc.sync.dma_start(
            out=wfcr[0:80, :, :],
            in_=_ap(wfc_d, 0, [[0, B], [C2 * 59, C2], [1, C2 * 59]]),
        )

        # gamma/beta: [g1(0:5) b1(5:10) g2(10:20) b2(20:30)]
        gb = singles.tile([1, 30], F32)
        nc.sync.dma_start(out=gb[:, 0:5], in_=g1_d[None, :])
        nc.sync.dma_start(out=gb[:, 5:10], in_=b1_d[None, :])
        nc.sync.dma_start(out=gb[:, 10:20], in_=g2_d[None, :])
        nc.sync.dma_start(out=gb[:, 20:30], in_=b2_d[None, :])

        # selectors / constants
        epst = singles.tile([1, 1], F32)
        nc.vector.memset(epst[:], BN_EPS)
        ones = singles.tile([128, 1], F32)
        nc.vector.memset(ones[:], 1.0)

        it64 = singles.tile([128, 64], I32)
        nc.gpsimd.iota(it64[:], pattern=[[-1, 64]], base=0, channel_multiplier=1)
        s64a = singles.tile([128, 64], F32)
        nc.vector.tensor_scalar(s64a[:], it64[:], 0.0, None, op0=OP.is_equal)
        s64b = singles.tile([128, 64], F32)
        nc.vector.tensor_scalar(s64b[:], it64[:], 64.0, None, op0=OP.is_equal)
        sel64 = singles.tile([128, 64], F32)
        nc.vector.tensor_add(sel64[:], s64a[:], s64b[:])

        it16 = singles.tile([128, 16], I32)
        nc.gpsimd.iota(it16[:], pattern=[[-5, 16]], base=0, channel_multiplier=1)
        s16a = singles.tile([128, 16], F32)
        nc.vector.tensor_scalar(s16a[:], it16[:], 0.0, None, op0=OP.is_ge)
        s16b = singles.tile([128, 16], F32)
        nc.vector.tensor_scalar(s16b[:], it16[:], 5.0, None, op0=OP.is_lt)
        sel16 = singles.tile([128, 16], F32)
        nc.vector.tensor_mul(sel16[:], s16a[:], s16b[:])

        # ---------------- conv1 ----------------
        ps1 = [psA.tile([64, HO], F32, tag=f"ps1_{co}", name=f"ps1_{co}")
               for co in range(C1)]

        for blk in range(NROUND // BLK):
            xs = xpool.tile([128, BLK, HO], F32, tag="xs")
            for tb in range(BLK):
                t = blk * BLK + tb
                for ks in range(2):
                    k = 2 * t + ks
                    r, c = k // KW, k % KW
                    nc.sync.dma_start(
                        out=xs[ks * 64:(ks + 1) * 64, tb, :],
                        in_=_ap(x_d, c * H + r,
                                [[H, WO], [W * H, B], [1, HO]]),
                    )
            HHO = HO // 2
            for co in range(C1):
                a = apool.tile([128, BLK, HO], F32, tag="a")
                for tb in range(BLK):
                    t = blk * BLK + tb
                    if co < 2:
                        nc.scalar.activation(
                            a[:, tb, :], xs[:, tb, :], AF.Abs,
                            bias=wneg[:, t, co:co + 1],
                        )
                    elif co == 2:
                        nc.scalar.activation(
                            a[:, tb, 0:HHO], xs[:, tb, 0:HHO], AF.Abs,
                            bias=wneg[:, t, co:co + 1],
                        )
                        nc.vector.tensor_scalar(
                            a[:, tb, HHO:HO], xs[:, tb, HHO:HO],
                            wpos[:, t, co:co + 1], None, op0=OP.subtract)
                    else:
                        nc.vector.tensor_scalar(
                            a[:, tb, :], xs[:, tb, :],
                            wpos[:, t, co:co + 1], None, op0=OP.subtract)
                if co == 2:
                    ai = a[:, :, HHO:HO].bitcast(I32)
                    nc.vector.tensor_scalar(ai, ai, 0x7FFFFFFF, None,
                                            op0=OP.bitwise_and)
                elif co > 2:
                    ai = a[:].bitcast(I32)
                    nc.vector.tensor_scalar(ai, ai, 0x7FFFFFFF, None,
                                            op0=OP.bitwise_and)
                for tb in range(BLK):
                    t = blk * BLK + tb
                    nc.tensor.matmul(
                        ps1[co][:], sel64[:, 0:64], a[:, tb, :],
                        start=(t == 0), stop=(t == NROUND - 1),
                    )

        # ---------------- evac + BN1 stats ----------------
        c1sb = singles.tile([64, C1, HO], F32)
        st1 = singles.tile([64, 2 * C1], F32)
        for co in range(C1):
            nc.scalar.activation(
                c1sb[:, co, :], ps1[co][:], AF.Copy,
                accum_out=st1[:, co:co + 1],
            )
            sq = scratch.tile([64, HO], F32, tag="sq")
            nc.scalar.activation(
                sq[:], ps1[co][:], AF.Square,
                accum_out=st1[:, C1 + co:C1 + co + 1],
            )
        pst1 = psB.tile([1, 2 * C1], F32, tag="stats")
        nc.tensor.matmul(pst1[:], ones[0:64, :], st1[:], start=True, stop=True)
        st1v = singles.tile([1, 2 * C1], F32)
        nc.vector.tensor_copy(st1v[:], pst1[:])

        ar1_in = dram.tile([1, 2 * C1], F32)
        ar1_out = dram.tile([1, 2 * C1], F32)
        nc.sync.dma_start(out=ar1_in[:], in_=st1v[:])
        nc.gpsimd.collective_compute(
            "AllReduce", OP.add,
            replica_groups=[list(range(N_CORES))],
            ins=[ar1_in[:].opt()], outs=[ar1_out[:].opt()],
        )
        gst1 = singles.tile([1, 2 * C1], F32)
        nc.sync.dma_start(out=gst1[:], in_=ar1_out[:])

        # affine params: y = relu(-scale*S + shift)
        m1 = singles.tile([1, C1], F32)
        nc.vector.tensor_scalar_mul(m1[:], gst1[:, 0:C1], 1.0 / NP1)
        v1 = singles.tile([1, C1], F32)
        nc.vector.tensor_scalar_mul(v1[:], gst1[:, C1:2 * C1], 1.0 / NP1)
        msq1 = singles.tile([1, C1], F32)
        nc.vector.tensor_mul(msq1[:], m1[:], m1[:])
        nc.vector.tensor_sub(v1[:], v1[:], msq1[:])
        std1 = singles.tile([1, C1], F32)
        nc.scalar.activation(std1[:], v1[:], AF.Sqrt, bias=epst[:])
        rstd1 = singles.tile([1, C1], F32)
        nc.vector.reciprocal(rstd1[:], std1[:])
        sc1 = singles.tile([1, C1], F32)
        nc.vector.tensor_mul(sc1[:], gb[:, 0:C1], rstd1[:])
        sh1 = singles.tile([1, C1], F32)
        nc.vector.tensor_mul(sh1[:], m1[:], sc1[:])
        nc.vector.tensor_add(sh1[:], gb[:, C1:2 * C1], sh1[:])
        nsc1 = singles.tile([1, C1], F32)
        nc.vector.tensor_scalar_mul(nsc1[:], sc1[:], -1.0)
        par1 = dram.tile([1, 2 * C1], F32)
        nc.sync.dma_start(out=par1[:, 0:C1], in_=nsc1[:])
        nc.sync.dma_start(out=par1[:, C1:2 * C1], in_=sh1[:])
        sc1v = singles.tile([128, 1], F32)
        nc.sync.dma_start(out=sc1v[0:80, :],
                          in_=_ap(par1[:], 0, [[0, 16], [1, C1]]))
        sh1v = singles.tile([128, 1], F32)
        nc.sync.dma_start(out=sh1v[0:80, :],
                          in_=_ap(par1[:], C1, [[0, 16], [1, C1]]))

        # ---------------- pool1 (min) + relayout ----------------
        q1 = singles.tile([64, C1, H2I], F32)
        nc.vector.tensor_reduce(
            q1[:], c1sb[:].rearrange("p c (i w) -> p c i w", w=4),
            axis=AX.X, op=OP.min,
        )
        r1 = singles.tile([128, 4, H2I], F32)
        for b in range(B):
            for w in range(2):
                for jj in range(4):
                    src_p = (4 * w + jj) * 8 + b
                    dst_p = b * 10 + w * 5
                    nc.sync.dma_start(
                        out=r1[dst_p:dst_p + 5, jj, :],
                        in_=q1[src_p:src_p + 1, :, :],
                    )
        p1raw = singles.tile([128, H2I], F32)
        nc.vector.tensor_reduce(p1raw[0:80], r1[0:80].transpose([0, 2, 1]),
                                axis=AX.X, op=OP.min)
        p1y = singles.tile([128, H2I], F32)
        nc.scalar.activation(p1y[0:80], p1raw[0:80], AF.Relu,
                             bias=sh1v[0:80], scale=sc1v[0:80])

        # ---------------- conv2 ----------------
        ps2 = [psB.tile([16, H2O], F32, tag="ps2", name=f"ps2_{c2}", bufs=2)
               for c2 in range(C2)]
        for c2 in range(C2):
            a2 = apool.tile([128, K2H, H2O], F32, tag="a2")
            for dr in range(K2H):
                if c2 < 4:
                    nc.scalar.activation(
                        a2[0:80, dr, :], p1y[0:80, dr:dr + H2O], AF.Abs,
                        bias=w2neg[0:80, c2, dr:dr + 1],
                    )
                else:
                    nc.vector.tensor_scalar(
                        a2[0:80, dr, :], p1y[0:80, dr:dr + H2O],
                        w2pos[0:80, c2, dr:dr + 1], None, op0=OP.subtract)
            if c2 >= 4:
                a2i = a2[0:80].bitcast(I32)
                nc.vector.tensor_scalar(a2i, a2i, 0x7FFFFFFF, None,
                                        op0=OP.bitwise_and)
            for dr in range(K2H):
                nc.tensor.matmul(
                    ps2[c2][:], sel16[0:80, :], a2[0:80, dr, :],
                    start=(dr == 0), stop=(dr == K2H - 1),
                )

        # evac + BN2 stats
        s2sb = singles.tile([16, C2, H2O], F32)
        st2 = singles.tile([16, 2 * C2], F32)
        for c2 in range(C2):
            nc.scalar.activation(
                s2sb[:, c2, :], ps2[c2][:], AF.Copy,
                accum_out=st2[:, c2:c2 + 1],
            )
            sq2 = scratch.tile([16, H2O], F32, tag="sq2")
            nc.scalar.activation(
                sq2[:], ps2[c2][:], AF.Square,
                accum_out=st2[:, C2 + c2:C2 + c2 + 1],
            )
        pst2 = psB.tile([1, 2 * C2], F32, tag="stats")
        nc.tensor.matmul(pst2[:], ones[0:16, :], st2[:], start=True, stop=True)
        st2v = singles.tile([1, 2 * C2], F32)
        nc.vector.tensor_copy(st2v[:], pst2[:])

        ar2_in = dram.tile([1, 2 * C2], F32)
        ar2_out = dram.tile([1, 2 * C2], F32)
        nc.sync.dma_start(out=ar2_in[:], in_=st2v[:])
        nc.gpsimd.collective_compute(
            "AllReduce", OP.add,
            replica_groups=[list(range(N_CORES))],
            ins=[ar2_in[:].opt()], outs=[ar2_out[:].opt()],
        )
        gst2 = singles.tile([1, 2 * C2], F32)
        nc.sync.dma_start(out=gst2[:], in_=ar2_out[:])

        m2 = singles.tile([1, C2], F32)
        nc.vector.tensor_scalar_mul(m2[:], gst2[:, 0:C2], 1.0 / NP2)
        v2 = singles.tile([1, C2], F32)
        nc.vector.tensor_scalar_mul(v2[:], gst2[:, C2:2 * C2], 1.0 / NP2)
        msq2 = singles.tile([1, C2], F32)
        nc.vector.tensor_mul(msq2[:], m2[:], m2[:])
        nc.vector.tensor_sub(v2[:], v2[:], msq2[:])
        std2 = singles.tile([1, C2], F32)
        nc.scalar.activation(std2[:], v2[:], AF.Sqrt, bias=epst[:])
        rstd2 = singles.tile([1, C2], F32)
        nc.vector.reciprocal(rstd2[:], std2[:])
        sc2 = singles.tile([1, C2], F32)
        nc.vector.tensor_mul(sc2[:], gb[:, 10:20], rstd2[:])
        sh2 = singles.tile([1, C2], F32)
        nc.vector.tensor_mul(sh2[:], m2[:], sc2[:])
        nc.vector.tensor_add(sh2[:], gb[:, 20:30], sh2[:])
        nsc2 = singles.tile([1, C2], F32)
        nc.vector.tensor_scalar_mul(nsc2[:], sc2[:], -1.0)
        par2 = dram.tile([1, 2 * C2], F32)
        nc.sync.dma_start(out=par2[:, 0:C2], in_=nsc2[:])
        nc.sync.dma_start(out=par2[:, C2:2 * C2], in_=sh2[:])
        sc2v = singles.tile([128, 1], F32)
        nc.sync.dma_start(out=sc2v[0:80, :],
                          in_=_ap(par2[:], 0, [[0, B], [1, C2]]))
        sh2v = singles.tile([128, 1], F32)
        nc.sync.dma_start(out=sh2v[0:80, :],
                          in_=_ap(par2[:], C2, [[0, B], [1, C2]]))

        # pool2-r (min) + relayout [16=(b,w),(c2,r)] -> [80=(b,c2),(r,w)]
        s2p = singles.tile([16, C2, 59], F32)
        nc.vector.tensor_reduce(
            s2p[:], s2sb[:].rearrange("p c (r w) -> p c r w", w=2),
            axis=AX.X, op=OP.min,
        )
        f2pre = singles.tile([128, 2, 59], F32)
        for b in range(B):
            for w in range(2):
                nc.sync.dma_start(
                    out=f2pre[b * 10:b * 10 + 10, w, :],
                    in_=s2p[b * 2 + w:b * 2 + w + 1, :, :],
                )
        f2raw = singles.tile([128, 59], F32)
        nc.vector.tensor_reduce(f2raw[0:80], f2pre[0:80].transpose([0, 2, 1]),
                                axis=AX.X, op=OP.min)
        f2y = singles.tile([128, 59], F32)
        nc.scalar.activation(f2y[0:80], f2raw[0:80], AF.Relu,
                             bias=sh2v[0:80], scale=sc2v[0:80])

        # ---------------- fc ----------------
        f2ap = f2y[:]
        f2b = bass.AP(tensor=f2ap.tensor, offset=f2ap.offset,
                      ap=[[f2ap.ap[0][0], 80], [0, C2], [1, 59]])
        d = singles.tile([128, C2, 59], F32)
        nc.vector.tensor_tensor(d[0:80], f2b, wfcr[0:80], op=OP.subtract)
        psfc = singles.tile([128, C2], F32)
        nc.vector.tensor_reduce(psfc[0:80], d[0:80], axis=AX.X, op=OP.add,
                                apply_absolute_value=True)
        nc.sync.dma_start(out=out_d[:, :], in_=psfc[0:80, :])

    nc.compile()
    return nc


_NC_CACHE = None


def _get_nc():
    global _NC_CACHE
    if _NC_CACHE is None:
        _NC_CACHE = build_kernel()
    return _NC_CACHE


def make_in_maps(inputs, w1, w2, w_fc, g1, b1, g2, b2):
    inputs = np.asarray(inputs, np.float32)
    # w1p[ks, t, co] = w1[co, 0, :, :].flat[2t+ks]
    w1sq = np.asarray(w1, np.float32).reshape(C1, K1)
    w1p = np.ascontiguousarray(
        w1sq.T.reshape(NROUND, 2, C1).transpose(1, 0, 2)).reshape(-1)
    # w2p[ci, c2, dr] = w2[c2, ci, dr, 0]
    w2p = np.ascontiguousarray(
        np.asarray(w2, np.float32)[..., 0].transpose(1, 0, 2)).reshape(-1)
    # wfcp[c2, c3, r] = w_fc[c3, c2*59+r, 0, 0]
    wfcp = np.ascontiguousarray(
        np.asarray(w_fc, np.float32)[:, :, 0, 0]
        .reshape(C2, C2, 59).transpose(1, 0, 2)).reshape(-1)
    in_maps = []
    for c in range(N_CORES):
        # [b, h, w] -> [b, w, h] so H is the contiguous axis
        shard = np.ascontiguousarray(
            inputs[c * B:(c + 1) * B, 0].transpose(0, 2, 1)).reshape(-1)
        in_maps.append({
            "x": shard,
            "w1p": w1p,
            "g1": np.ascontiguousarray(g1, np.float32),
            "b1": np.ascontiguousarray(b1, np.float32),
            "w2p": w2p,
            "g2": np.ascontiguousarray(g2, np.float32),
            "b2": np.ascontiguousarray(b2, np.float32),
            "wfcp": wfcp,
        })
    return in_maps


def postprocess(outs, g3, b3):
    z = np.concatenate(
        [-np.asarray(o).reshape(B, C2, C2).sum(axis=1) for o in outs], axis=0
    ).astype(np.float32)
    m = z.mean(axis=0)
    v = z.var(axis=0)
    zn = (z - m) / np.sqrt(v + BN_EPS)
    out = zn * np.asarray(g3, np.float32) + np.asarray(b3, np.float32)
    return out.astype(np.float32)


def kernel(inputs, w1, g1, b1, w2, g2, b2, w_fc, g3, b3, _results=None):
    nc = _get_nc()
    in_maps = make_in_maps(inputs, w1, w2, w_fc, g1, b1, g2, b2)
    res = bass_utils.run_bass_kernel_spmd(nc, in_maps, core_ids=list(range(N_CORES)))
    if _results is not None:
        _results.append(res)
    outs = [r["out"] for r in res.results]
    return postprocess(outs, g3, b3)


if __name__ == "__main__":
    nc = build_kernel()
    print("build + compile OK")


# revision 10
# speedup vs baseline: 146.2054x; 146.2054x over previous
"""AdderNet CNN (nn_CNN_73306501808283) on 8 Trainium2 NeuronCores.

Data-parallel over batch (64 -> 8 per core). Full-batch BN statistics via
two tiny in-kernel AllReduces. Final channel-sum + negate + BN3 on host.

Per-core layout:
  conv1: 30 rounds of k-pairs; partitions = (ks=2, j=8, b=8) = 128.
    X_shift[(ks,j,b), i] = x[b, i+r(k), j+c(k)], k = 2t+ks, k = r*5+c.
    A = |X_shift - w1[co,k]|  (ScalarE Abs-with-bias / VectorE tensor_scalar)
    k-sum on TensorE: selector lhsT[128,64] passes (j,b) through, PSUM
    accumulates over 30 rounds -> psum_co[64=(j,b), 488=i] (POSITIVE sums;
    real conv out = -psum).
  BN sums fused into PSUM evacuation via activation(accum_out); pools are
  MIN-pools on the raw positive sums (BN scale > 0 because g == 1, and conv
  out = -sum, so the affine+relu is monotonically decreasing in the raw sum).
"""

import sys
from contextlib import ExitStack

import numpy as np

if "/opt/trn_rl_repo" not in sys.path:
    sys.path.insert(0, "/opt/trn_rl_repo")

import concourse.bass as bass
import concourse.tile as tile
from concourse import bacc, mybir
from concourse import bass_utils

F32 = mybir.dt.float32
I32 = mybir.dt.int32
AF = mybir.ActivationFunctionType
OP = mybir.AluOpType
AX = mybir.AxisListType

N_CORES = 8
B = 8          # images per core
H, W = 499, 12
KW = 5
C1 = 5         # conv1 out channels
HO, WO = 488, 8
K1 = 60
NROUND = 30    # k pairs
BLK = 3        # rounds per X_shift/A block
C2 = 10
K2H = 5
H2I = 122      # H after pool1
H2O = 118
NP1 = 64 * HO * WO
NP2 = 64 * H2O * 2
BN_EPS = 1e-5


def _ap(t_ap, offset, dims):
    return bass.AP(tensor=t_ap.tensor, offset=offset, ap=[list(d) for d in dims])


def _do_allreduce(nc, ar_in, ar_out, mock_cc):
    if mock_cc:
        nc.sync.dma_start(out=ar_out[:], in_=ar_in[:])
    else:
        nc.gpsimd.collective_compute(
            "AllReduce", OP.add,
            replica_groups=[list(range(N_CORES))],
            ins=[ar_in[:].opt()], outs=[ar_out[:].opt()],
        )


def build_kernel(loop_n=1, mock_cc=False):
    nc = bacc.Bacc(
        "TRN2",
        target_bir_lowering=False,
        debug=False,
        enable_asserts=True,
        num_devices=N_CORES,
    )

    # x is host-transposed to [b, w, h]; w1p/w2p/wfcp are host-permuted tables
    x_d = nc.dram_tensor("x", [B * W * H], F32, kind="ExternalInput").ap()
    w1_d = nc.dram_tensor("w1p", [2 * NROUND * C1], F32, kind="ExternalInput").ap()
    g1_d = nc.dram_tensor("g1", [C1], F32, kind="ExternalInput").ap()
    b1_d = nc.dram_tensor("b1", [C1], F32, kind="ExternalInput").ap()
    w2_d = nc.dram_tensor("w2p", [C1 * C2 * K2H], F32, kind="ExternalInput").ap()
    g2_d = nc.dram_tensor("g2", [C2], F32, kind="ExternalInput").ap()
    b2_d = nc.dram_tensor("b2", [C2], F32, kind="ExternalInput").ap()
    wfc_d = nc.dram_tensor("wfcp", [C2 * C2 * 59], F32, kind="ExternalInput").ap()
    out_d = nc.dram_tensor("out", [80, C2], F32, kind="ExternalOutput").ap()

    with tile.TileContext(nc) as tc, ExitStack() as ctx:
        singles = ctx.enter_context(tc.tile_pool(name="singles", bufs=1))
        xpool = ctx.enter_context(tc.tile_pool(name="xshift", bufs=3))
        apool = ctx.enter_context(tc.tile_pool(name="adiff", bufs=8))
        psA = ctx.enter_context(tc.tile_pool(name="psA", bufs=1, space="PSUM"))
        psB = ctx.enter_context(tc.tile_pool(name="psB", bufs=1, space="PSUM"))
        scratch = ctx.enter_context(tc.tile_pool(name="scratch", bufs=2))
        dram = ctx.enter_context(tc.tile_pool(name="dram", bufs=1, space="DRAM"))

        # ---------------- one-time setup ----------------
        # conv1 bias tables: wpos[p=(ks,j,b), t, co] = w1[co, 2t+ks]
        wpos = singles.tile([128, NROUND, C1], F32)
        for ks in range(2):
            nc.sync.dma_start(
                out=wpos[ks * 64:(ks + 1) * 64, :, :],
                in_=_ap(w1_d, ks * NROUND * C1, [[0, 64], [1, NROUND * C1]]),
            )
        wneg = singles.tile([128, NROUND, C1], F32)
        nc.vector.tensor_scalar_mul(wneg[:], wpos[:], -1.0)

        # conv2 tables: p=(b,w,ci) 80 partitions; free (c2, dr)
        w2pos = singles.tile([128, C2, K2H], F32)
        nc.sync.dma_start(
            out=w2pos[0:80, :, :],
            in_=_ap(w2_d, 0, [[0, 16], [C2 * K2H, C1], [1, C2 * K2H]]),
        )
        w2neg = singles.tile([128, C2, K2H], F32)
        nc.vector.tensor_scalar_mul(w2neg[0:80], w2pos[0:80], -1.0)

        # fc weights: p=(b,c2) 80; free (c3, r)
        wfcr = singles.tile([128, C2, 59], F32)
        nc.sync.dma_start(
            out=wfcr[0:80, :, :],
            in_=_ap(wfc_d, 0, [[0, B], [C2 * 59, C2], [1, C2 * 59]]),
        )

        # gamma/beta: [g1(0:5) b1(5:10) g2(10:20) b2(20:30)]
        gb = singles.tile([1, 30], F32)
        nc.sync.dma_start(out=gb[:, 0:5], in_=g1_d[None, :])
        nc.sync.dma_start(out=gb[:, 5:10], in_=b1_d[None, :])
        nc.sync.dma_start(out=gb[:, 10:20], in_=g2_d[None, :])
        nc.sync.dma_start(out=gb[:, 20:30], in_=b2_d[None, :])

        # selectors / constants
        epst = singles.tile([1, 1], F32)
        nc.vector.memset(epst[:], BN_EPS)
        ones = singles.tile([128, 1], F32)
        nc.vector.memset(ones[:], 1.0)

        it64 = singles.tile([128, 64], I32)
        nc.gpsimd.iota(it64[:], pattern=[[-1, 64]], base=0, channel_multiplier=1)
        s64a = singles.tile([128, 64], F32)
        nc.vector.tensor_scalar(s64a[:], it64[:], 0.0, None, op0=OP.is_equal)
        s64b = singles.tile([128, 64], F32)
        nc.vector.tensor_scalar(s64b[:], it64[:], 64.0, None, op0=OP.is_equal)
        sel64 = singles.tile([128, 64], F32)
        nc.vector.tensor_add(sel64[:], s64a[:], s64b[:])

        it16 = singles.tile([128, 16], I32)
        nc.gpsimd.iota(it16[:], pattern=[[-5, 16]], base=0, channel_multiplier=1)
        s16a = singles.tile([128, 16], F32)
        nc.vector.tensor_scalar(s16a[:], it16[:], 0.0, None, op0=OP.is_ge)
        s16b = singles.tile([128, 16], F32)
        nc.vector.tensor_scalar(s16b[:], it16[:], 5.0, None, op0=OP.is_lt)
        sel16 = singles.tile([128, 16], F32)
        nc.vector.tensor_mul(sel16[:], s16a[:], s16b[:])

        # ---------------- conv1 ----------------
        loop_ctx = tc.For_i(0, loop_n, 1) if loop_n > 1 else None
        if loop_ctx is not None:
            loop_ctx.__enter__()
        ps1 = [psA.tile([64, HO], F32, tag=f"ps1_{co}", name=f"ps1_{co}")
               for co in range(C1)]

        for blk in range(NROUND // BLK):
            xs = xpool.tile([128, BLK, HO], F32, tag="xs")
            for tb in range(BLK):
                t = blk * BLK + tb
                for ks in range(2):
                    k = 2 * t + ks
                    r, c = k // KW, k % KW
                    nc.sync.dma_start(
                        out=xs[ks * 64:(ks + 1) * 64, tb, :],
                        in_=_ap(x_d, c * H + r,
                                [[H, WO], [W * H, B], [1, HO]]),
                    )
            HHO = HO // 2
            for co in range(C1):
                a = apool.tile([128, BLK, HO], F32, tag="a")
                for tb in range(BLK):
                    t = blk * BLK + tb
                    if co < 2:
                        nc.scalar.activation(
                            a[:, tb, :], xs[:, tb, :], AF.Abs,
                            bias=wneg[:, t, co:co + 1],
                        )
                    elif co == 2:
                        nc.scalar.activation(
                            a[:, tb, 0:HHO], xs[:, tb, 0:HHO], AF.Abs,
                            bias=wneg[:, t, co:co + 1],
                        )
                        nc.vector.tensor_scalar(
                            a[:, tb, HHO:HO], xs[:, tb, HHO:HO],
                            wpos[:, t, co:co + 1], None, op0=OP.subtract)
                    else:
                        nc.vector.tensor_scalar(
                            a[:, tb, :], xs[:, tb, :],
                            wpos[:, t, co:co + 1], None, op0=OP.subtract)
                if co == 2:
                    ai = a[:, :, HHO:HO].bitcast(I32)
                    nc.vector.tensor_scalar(ai, ai, 0x7FFFFFFF, None,
                                            op0=OP.bitwise_and)
                elif co > 2:
                    ai = a[:].bitcast(I32)
                    nc.vector.tensor_scalar(ai, ai, 0x7FFFFFFF, None,
                                            op0=OP.bitwise_and)
                for tb in range(BLK):
                    t = blk * BLK + tb
                    nc.tensor.matmul(
                        ps1[co][:], sel64[:, 0:64], a[:, tb, :],
                        start=(t == 0), stop=(t == NROUND - 1),
                    )

        # ---------------- evac + BN1 stats ----------------
        c1sb = singles.tile([64, C1, HO], F32)
        st1 = singles.tile([64, 2 * C1], F32)
        for co in range(C1):
            nc.scalar.activation(
                c1sb[:, co, :], ps1[co][:], AF.Copy,
                accum_out=st1[:, co:co + 1],
            )
            sq = scratch.tile([64, HO], F32, tag="sq")
            nc.scalar.activation(
                sq[:], ps1[co][:], AF.Square,
                accum_out=st1[:, C1 + co:C1 + co + 1],
            )
        pst1 = psB.tile([1, 2 * C1], F32, tag="stats")
        nc.tensor.matmul(pst1[:], ones[0:64, :], st1[:], start=True, stop=True)
        st1v = singles.tile([1, 2 * C1], F32)
        nc.vector.tensor_copy(st1v[:], pst1[:])

        ar1_in = dram.tile([1, 2 * C1], F32)
        ar1_out = dram.tile([1, 2 * C1], F32)
        nc.sync.dma_start(out=ar1_in[:], in_=st1v[:])
        _do_allreduce(nc, ar1_in, ar1_out, mock_cc)
        gst1 = singles.tile([1, 2 * C1], F32)
        nc.sync.dma_start(out=gst1[:], in_=ar1_out[:])

        # affine params: y = relu(-scale*S + shift)
        m1 = singles.tile([1, C1], F32)
        nc.vector.tensor_scalar_mul(m1[:], gst1[:, 0:C1], 1.0 / NP1)
        v1 = singles.tile([1, C1], F32)
        nc.vector.tensor_scalar_mul(v1[:], gst1[:, C1:2 * C1], 1.0 / NP1)
        msq1 = singles.tile([1, C1], F32)
        nc.vector.tensor_mul(msq1[:], m1[:], m1[:])
        nc.vector.tensor_sub(v1[:], v1[:], msq1[:])
        std1 = singles.tile([1, C1], F32)
        nc.scalar.activation(std1[:], v1[:], AF.Sqrt, bias=epst[:])
        rstd1 = singles.tile([1, C1], F32)
        nc.vector.reciprocal(rstd1[:], std1[:])
        sc1 = singles.tile([1, C1], F32)
        nc.vector.tensor_mul(sc1[:], gb[:, 0:C1], rstd1[:])
        sh1 = singles.tile([1, C1], F32)
        nc.vector.tensor_mul(sh1[:], m1[:], sc1[:])
        nc.vector.tensor_add(sh1[:], gb[:, C1:2 * C1], sh1[:])
        nsc1 = singles.tile([1, C1], F32)
        nc.vector.tensor_scalar_mul(nsc1[:], sc1[:], -1.0)
        par1 = dram.tile([1, 2 * C1], F32)
        nc.sync.dma_start(out=par1[:, 0:C1], in_=nsc1[:])
        nc.sync.dma_start(out=par1[:, C1:2 * C1], in_=sh1[:])
        sc1v = singles.tile([128, 1], F32)
        nc.sync.dma_start(out=sc1v[0:80, :],
                          in_=_ap(par1[:], 0, [[0, 16], [1, C1]]))
        sh1v = singles.tile([128, 1], F32)
        nc.sync.dma_start(out=sh1v[0:80, :],
                          in_=_ap(par1[:], C1, [[0, 16], [1, C1]]))

        # ---------------- pool1 (min) + relayout ----------------
        q1 = singles.tile([64, C1, H2I], F32)
        nc.vector.tensor_reduce(
            q1[:], c1sb[:].rearrange("p c (i w) -> p c i w", w=4),
            axis=AX.X, op=OP.min,
        )
        r1 = singles.tile([128, 4, H2I], F32)
        for b in range(B):
            for w in range(2):
                for jj in range(4):
                    src_p = (4 * w + jj) * 8 + b
                    dst_p = b * 10 + w * 5
                    nc.sync.dma_start(
                        out=r1[dst_p:dst_p + 5, jj, :],
                        in_=q1[src_p:src_p + 1, :, :],
                    )
        p1raw = singles.tile([128, H2I], F32)
        nc.vector.tensor_reduce(p1raw[0:80], r1[0:80].transpose([0, 2, 1]),
                                axis=AX.X, op=OP.min)
        p1y = singles.tile([128, H2I], F32)
        nc.scalar.activation(p1y[0:80], p1raw[0:80], AF.Relu,
                             bias=sh1v[0:80], scale=sc1v[0:80])

        # ---------------- conv2 ----------------
        ps2 = [psB.tile([16, H2O], F32, tag="ps2", name=f"ps2_{c2}", bufs=2)
               for c2 in range(C2)]
        for c2 in range(C2):
            a2 = apool.tile([128, K2H, H2O], F32, tag="a2")
            for dr in range(K2H):
                if c2 < 4:
                    nc.scalar.activation(
                        a2[0:80, dr, :], p1y[0:80, dr:dr + H2O], AF.Abs,
                        bias=w2neg[0:80, c2, dr:dr + 1],
                    )
                else:
                    nc.vector.tensor_scalar(
                        a2[0:80, dr, :], p1y[0:80, dr:dr + H2O],
                        w2pos[0:80, c2, dr:dr + 1], None, op0=OP.subtract)
            if c2 >= 4:
                a2i = a2[0:80].bitcast(I32)
                nc.vector.tensor_scalar(a2i, a2i, 0x7FFFFFFF, None,
                                        op0=OP.bitwise_and)
            for dr in range(K2H):
                nc.tensor.matmul(
                    ps2[c2][:], sel16[0:80, :], a2[0:80, dr, :],
                    start=(dr == 0), stop=(dr == K2H - 1),
                )

        # evac + BN2 stats
        s2sb = singles.tile([16, C2, H2O], F32)
        st2 = singles.tile([16, 2 * C2], F32)
        for c2 in range(C2):
            nc.scalar.activation(
                s2sb[:, c2, :], ps2[c2][:], AF.Copy,
                accum_out=st2[:, c2:c2 + 1],
            )
            sq2 = scratch.tile([16, H2O], F32, tag="sq2")
            nc.scalar.activation(
                sq2[:], ps2[c2][:], AF.Square,
                accum_out=st2[:, C2 + c2:C2 + c2 + 1],
            )
        pst2 = psB.tile([1, 2 * C2], F32, tag="stats")
        nc.tensor.matmul(pst2[:], ones[0:16, :], st2[:], start=True, stop=True)
        st2v = singles.tile([1, 2 * C2], F32)
        nc.vector.tensor_copy(st2v[:], pst2[:])

        ar2_in = dram.tile([1, 2 * C2], F32)
        ar2_out = dram.tile([1, 2 * C2], F32)
        nc.sync.dma_start(out=ar2_in[:], in_=st2v[:])
        _do_allreduce(nc, ar2_in, ar2_out, mock_cc)
        gst2 = singles.tile([1, 2 * C2], F32)
        nc.sync.dma_start(out=gst2[:], in_=ar2_out[:])

        m2 = singles.tile([1, C2], F32)
        nc.vector.tensor_scalar_mul(m2[:], gst2[:, 0:C2], 1.0 / NP2)
        v2 = singles.tile([1, C2], F32)
        nc.vector.tensor_scalar_mul(v2[:], gst2[:, C2:2 * C2], 1.0 / NP2)
        msq2 = singles.tile([1, C2], F32)
        nc.vector.tensor_mul(msq2[:], m2[:], m2[:])
        nc.vector.tensor_sub(v2[:], v2[:], msq2[:])
        std2 = singles.tile([1, C2], F32)
        nc.scalar.activation(std2[:], v2[:], AF.Sqrt, bias=epst[:])
        rstd2 = singles.tile([1, C2], F32)
        nc.vector.reciprocal(rstd2[:], std2[:])
        sc2 = singles.tile([1, C2], F32)
        nc.vector.tensor_mul(sc2[:], gb[:, 10:20], rstd2[:])
        sh2 = singles.tile([1, C2], F32)
        nc.vector.tensor_mul(sh2[:], m2[:], sc2[:])
        nc.vector.tensor_add(sh2[:], gb[:, 20:30], sh2[:])
        nsc2 = singles.tile([1, C2], F32)
        nc.vector.tensor_scalar_mul(nsc2[:], sc2[:], -1.0)
        par2 = dram.tile([1, 2 * C2], F32)
        nc.sync.dma_start(out=par2[:, 0:C2], in_=nsc2[:])
        nc.sync.dma_start(out=par2[:, C2:2 * C2], in_=sh2[:])
        sc2v = singles.tile([128, 1], F32)
        nc.sync.dma_start(out=sc2v[0:80, :],
                          in_=_ap(par2[:], 0, [[0, B], [1, C2]]))
        sh2v = singles.tile([128, 1], F32)
        nc.sync.dma_start(out=sh2v[0:80, :],
                          in_=_ap(par2[:], C2, [[0, B], [1, C2]]))

        # pool2-r (min) + relayout [16=(b,w),(c2,r)] -> [80=(b,c2),(r,w)]
        s2p = singles.tile([16, C2, 59], F32)
        nc.vector.tensor_reduce(
            s2p[:], s2sb[:].rearrange("p c (r w) -> p c r w", w=2),
            axis=AX.X, op=OP.min,
        )
        f2pre = singles.tile([128, 2, 59], F32)
        for b in range(B):
            for w in range(2):
                nc.sync.dma_start(
                    out=f2pre[b * 10:b * 10 + 10, w, :],
                    in_=s2p[b * 2 + w:b * 2 + w + 1, :, :],
                )
        f2raw = singles.tile([128, 59], F32)
        nc.vector.tensor_reduce(f2raw[0:80], f2pre[0:80].transpose([0, 2, 1]),
                                axis=AX.X, op=OP.min)
        f2y = singles.tile([128, 59], F32)
        nc.scalar.activation(f2y[0:80], f2raw[0:80], AF.Relu,
                             bias=sh2v[0:80], scale=sc2v[0:80])

        # ---------------- fc ----------------
        f2ap = f2y[:]
        f2b = bass.AP(tensor=f2ap.tensor, offset=f2ap.offset,
                      ap=[[f2ap.ap[0][0], 80], [0, C2], [1, 59]])
        d = singles.tile([128, C2, 59], F32)
        nc.vector.tensor_tensor(d[0:80], f2b, wfcr[0:80], op=OP.subtract)
        psfc = singles.tile([128, C2], F32)
        nc.vector.tensor_reduce(psfc[0:80], d[0:80], axis=AX.X, op=OP.add,
                                apply_absolute_value=True)
        nc.sync.dma_start(out=out_d[:, :], in_=psfc[0:80, :])
        if loop_ctx is not None:
            loop_ctx.__exit__(None, None, None)

    nc.compile()
    return nc


_NC_CACHE = None


def _get_nc():
    global _NC_CACHE
    if _NC_CACHE is None:
        _NC_CACHE = build_kernel()
    return _NC_CACHE


def make_in_maps(inputs, w1, w2, w_fc, g1, b1, g2, b2):
    inputs = np.asarray(inputs, np.float32)
    # w1p[ks, t, co] = w1[co, 0, :, :].flat[2t+ks]
    w1sq = np.asarray(w1, np.float32).reshape(C1, K1)
    w1p = np.ascontiguousarray(
        w1sq.T.reshape(NROUND, 2, C1).transpose(1, 0, 2)).reshape(-1)
    # w2p[ci, c2, dr] = w2[c2, ci, dr, 0]
    w2p = np.ascontiguousarray(
        np.asarray(w2, np.float32)[..., 0].transpose(1, 0, 2)).reshape(-1)
    # wfcp[c2, c3, r] = w_fc[c3, c2*59+r, 0, 0]
    wfcp = np.ascontiguousarray(
        np.asarray(w_fc, np.float32)[:, :, 0, 0]
        .reshape(C2, C2, 59).transpose(1, 0, 2)).reshape(-1)
    in_maps = []
    for c in range(N_CORES):
        # [b, h, w] -> [b, w, h] so H is the contiguous axis
        shard = np.ascontiguousarray(
            inputs[c * B:(c + 1) * B, 0].transpose(0, 2, 1)).reshape(-1)
        in_maps.append({
            "x": shard,
            "w1p": w1p,
            "g1": np.ascontiguousarray(g1, np.float32),
            "b1": np.ascontiguousarray(b1, np.float32),
            "w2p": w2p,
            "g2": np.ascontiguousarray(g2, np.float32),
            "b2": np.ascontiguousarray(b2, np.float32),
            "wfcp": wfcp,
        })
    return in_maps


def postprocess(outs, g3, b3):
    z = np.concatenate(
        [-np.asarray(o).reshape(B, C2, C2).sum(axis=1) for o in outs], axis=0
    ).astype(np.float32)
    m = z.mean(axis=0)
    v = z.var(axis=0)
    zn = (z - m) / np.sqrt(v + BN_EPS)
    out = zn * np.asarray(g3, np.float32) + np.asarray(b3, np.float32)
    return out.astype(np.float32)


def kernel(inputs, w1, g1, b1, w2, g2, b2, w_fc, g3, b3, _results=None):
    nc = _get_nc()
    in_maps = make_in_maps(inputs, w1, w2, w_fc, g1, b1, g2, b2)
    res = bass_utils.run_bass_kernel_spmd(nc, in_maps, core_ids=list(range(N_CORES)))
    if _results is not None:
        _results.append(res)
    outs = [r["out"] for r in res.results]
    return postprocess(outs, g3, b3)


if __name__ == "__main__":
    nc = build_kernel()
    print("build + compile OK")
